# revision 28
# baseline (speedup 1.0000x reference)
"""Trainium2 Bass kernel for nn_BoundaryDetectionLoss.

Computes, for start/end (probs, targets) pairs of shape (64, 131072):
    w   = 1 + exp(-dist_to_nearest_boundary / 5)     (distance transform)
    bce = (1-z)*x + (1+z)*softplus(-x)               (pos_weight = 2)
    loss = mean(bce * w)   per pair; total = (start_loss + end_loss)/2

Identity used on device (g = softplus(+x), e = exp(-dist/5), z*e == z):
    bce*w = g*(1 + e + 2z) - 4*z*x
so with host-staged z2 = 2*z (zero-padded halo) and e2 = 2e from the
decayed-max scans:
    sum(bce*w) = sum(g) + 0.5*sum(g*e2) + sum(z2*g) - 2*sum(z2*x)

Device algorithm (per core, data-parallel over 8 rows of B=64):
  - e2[t] = 2*exp(-dist[t]/5) as a decayed-max field with two DVE
    tensor_tensor_scan passes (op0=mult by a=exp(-1/5), op1=max), 128-element
    halo per tile (contributions beyond ~84 positions underflow below fp16).
    Scans are DVE-only: TensorTensorScanArith is not a legal GPSIMD opcode.
  - g = softplus(x) = ln(1+exp(x)) on ACT (Exp+Ln share one LUT set; walrus
    has no softplus set); the Ln's accum_out gives sum(g) per partition free.
  - Dots sum(z2*g), sum(z2*x), sum(e2*g) on the PE via 128-wide block
    matmuls accumulating lhsT^T @ rhs into PSUM; host sums the diagonals.
  - Inputs staged fp16 by the host (pure dtype conversion + padding):
    halves HBM traffic vs f32 and feeds the PE directly.
"""

import sys

for _p in ("/opt/trn_rl_repo", "/root/.axon_site/_ro/trn_rl_repo"):
    if _p not in sys.path:
        sys.path.append(_p)

import numpy as np

# ---------------------------------------------------------------- config
B_FULL = 64
T_FULL = 131072
N_CORES = 8
ROWS = B_FULL // N_CORES  # 8 rows per core
DECAY = float(np.float16(np.exp(np.float32(-0.2))))  # a = exp(-1/5) in fp16
# two fp16 DECAY values bit-packed as one f32 (memset the const tile at
# half the DVE cycles by writing f32 pairs)
DECAY_PAIR = float(
    np.frombuffer(np.array([DECAY, DECAY], np.float16).tobytes(), np.float32)[0]
)


class Cfg:
    def __init__(self, rows=8, chunks=16, j_tiles=2, tile_len=4096, halo=64,
                 z_dt="float8e4", x_dt="float8e4", e2_dt="float8e4",
                 const_mode="broadcast", texp_bufs=2):
        self.rows = rows
        self.chunks = chunks
        self.j_tiles = j_tiles
        self.tile_len = tile_len
        self.halo = halo
        self.chunk_len = j_tiles * tile_len
        self.T = chunks * self.chunk_len
        self.parts = rows * chunks
        assert self.parts <= 128
        self.blk = 128
        self.n_blk = tile_len // self.blk
        assert halo <= tile_len
        self.z_dt = z_dt
        self.x_dt = x_dt
        self.e2_dt = e2_dt
        self.const_mode = const_mode
        self.texp_bufs = texp_bufs
        self.n_tiles = 2 * j_tiles


PROD_CFG = Cfg()
PAIRS = (("start_probs", "start_targets"), ("end_probs", "end_targets"))


def _build_body(nc, tc, cfg, dram_in, acc, psums_z, psums_e,
                const_v, pools, bass, mybir):
    f16 = mybir.dt.float16
    AF = mybir.ActivationFunctionType
    OP = mybir.AluOpType
    zpool, gpool, epool, e2pool, tpool = pools
    P, TL, H = cfg.parts, cfg.tile_len, cfg.halo
    W = TL + 2 * H
    zdt = getattr(mybir.dt, cfg.z_dt)
    xdt = getattr(mybir.dt, cfg.x_dt)
    e2dt = getattr(mybir.dt, cfg.e2_dt)
    Tp = cfg.T + 2 * H  # padded row length
    nt = cfg.n_tiles
    BLK = cfg.blk

    # Tile 0 is the pipeline head: its loads/softplus/scan are split into
    # pieces (SEPARATE tiles — Tile tracks hazards per tile, not per range)
    # so the DVE scan chain and the ACT chain both start as early as
    # possible. Piece boundaries are block-aligned for the PE slices.
    ZW0_CUTS = (0, H + 4 * BLK, H + 16 * BLK, W)
    GX0_CUTS = (0, TL // 2, TL)

    # ---- phase 1: DMA loads + ACT softplus, tile-major (zw first: the DVE
    # scan chain is the critical path and consumes zw earliest)
    tiles = []  # per tile: dict(zw=[(lo,hi,tile)], gx=[(tlo,thi,tile,acc_col)])
    for pi, (px, pz) in enumerate(PAIRS):
        xd, zd = dram_in[px], dram_in[pz]
        x4 = xd[:].rearrange(
            "r (c j f) -> (r c) j f", c=cfg.chunks, j=cfg.j_tiles
        )
        for j in range(cfg.j_tiles):
            ti = pi * cfg.j_tiles + j
            zw_cuts = ZW0_CUTS if ti == 0 else (0, W)
            gx_cuts = GX0_CUTS if ti == 0 else (0, TL)

            zw = []
            for si in range(len(zw_cuts) - 1):
                lo, hi = zw_cuts[si], zw_cuts[si + 1]
                zt = zpool.tile([P, hi - lo], zdt, tag=f"zw{min(ti,1)}{si}",
                                name=f"zw{min(ti,1)}{si}")
                zw.append((lo, hi, zt))

            # gx piece = [g | x] halves: DMA x into the right half, ACT
            # writes g = softplus(x) = ln(1 + exp(x)) into the left half,
            # so one 256-wide PE moving operand covers both z2@g and z2@x.
            # (No softplus LUT set exists in walrus; Exp+Ln share one set.)
            gx = []
            for si in range(len(gx_cuts) - 1):
                tlo, thi = gx_cuts[si], gx_cuts[si + 1]
                n = thi - tlo
                gt = gpool.tile([P, 2 * n], xdt, tag=f"gx{min(ti,1)}{si}",
                                name=f"gx{min(ti,1)}{si}")
                gx.append((tlo, thi, gt))

            # DMA issue order: first zw piece, then first x piece (unblocks
            # the ACT chain), then the rest
            def _dma_z(si):
                lo, hi, zt = zw[si]
                zwin = bass.AP(
                    zd,
                    j * TL + lo,
                    [[Tp, cfg.rows], [cfg.chunk_len, cfg.chunks],
                     [1, hi - lo]],
                )
                nc.sync.dma_start(zt[:], zwin)

            def _dma_x(si):
                tlo, thi, gt = gx[si]
                n = thi - tlo
                nc.sync.dma_start(gt[:, n : 2 * n], x4[:, j, tlo:thi])

            for si in range(len(zw)):
                _dma_z(si)
            for si in range(len(gx)):
                _dma_x(si)

            for si, (tlo, thi, gt) in enumerate(gx):
                n = thi - tlo
                texp = tpool.tile([P, n], f16, tag=f"texp{min(ti,1)}{si}",
                                  name=f"texp{min(ti,1)}{si}")
                nc.scalar.activation(texp[:], gt[:, n : 2 * n], AF.Exp)
                col = ti if si == 0 else nt + si - 1  # extra accum cols
                nc.scalar.activation(
                    gt[:, 0:n], texp[:], AF.Ln, bias=1.0,
                    accum_out=acc[:, col : col + 1],
                )
            tiles.append(dict(pi=pi, j=j, zw=zw, gx=gx))

    # ---- phase 2: DVE scans (fwd full window in chained piece segments,
    # rev only [H, W) reversed). Scans carry at most one ISA sync wait;
    # _split_multiwaits moves any extras Tile attaches onto same-engine
    # NoOps. The LAST tile's reverse scan lands in three chained segment
    # tiles so its e-matmuls (the tail of the whole kernel) unblock chunk
    # by chunk.
    if cfg.const_mode == "broadcast":
        cb = lambda n: const_v[:].broadcast_to((P, n))  # noqa: E731
    else:
        cb = lambda n: const_v[:, 0:n]  # noqa: E731 (value-constant tile)
    e2s = []
    for ti, t in enumerate(tiles):
        ef = epool.tile([P, W], f16, tag="ef")
        prev_hi = None
        for lo, hi, zt in t["zw"]:
            init = 0.0 if prev_hi is None else ef[:, lo - 1 : lo]
            nc.vector.tensor_tensor_scan(
                ef[:, lo:hi], cb(hi - lo), zt[:], init, OP.mult, OP.max)
            prev_hi = hi
        if ti == nt - 1:
            m2 = H + TL // 2
            m1 = H + TL // 4
            segs = []
            prev = None
            for si, (lo, hi) in enumerate(((m2, W), (m1, m2), (H, m1))):
                st = e2pool.tile([P, hi - lo], e2dt, tag=f"e2s{si}",
                                 name=f"e2s{si}")
                init = 0.0 if prev is None else prev[:, 0:1]
                nc.vector.tensor_tensor_scan(
                    st[:, ::-1], cb(hi - lo),
                    ef[:, hi - 1 : lo - 1 : -1], init, OP.mult, OP.max
                )
                segs.append((lo, hi, st))
                prev = st
            e2s.append(segs)
        else:
            e2 = e2pool.tile([P, W], e2dt, tag="e2")
            nc.vector.tensor_tensor_scan(
                e2[:, W - 1 : H - 1 : -1], cb(W - H),
                ef[:, W - 1 : H - 1 : -1], 0.0, OP.mult, OP.max
            )
            e2s.append((H, W, e2))

    # ---- phase 3: PE matmuls. z-mms of a tile depend only on (zw, gx);
    # e-mms additionally on that tile's rev scan. Order z(0), z(1), e(0),
    # z(2), e(1), z(3), e(2), e(3) keeps the PE fed while scans complete.
    def pick(pieces, lo):
        for plo, phi, pt in pieces:
            if plo <= lo < phi:
                return plo, pt
        raise AssertionError(f"no piece for {lo}")

    def rhs_for(t, b, g_only):
        tpos = b * BLK
        tlo, gt = pick(t["gx"], tpos)
        n = gt.shape[1] // 2
        o = tpos - tlo
        if g_only:
            return gt[:, o : o + BLK]
        g3 = gt[:].rearrange("p (g f) -> p g f", g=2)
        return g3[:, :, o : o + BLK]

    def z_mms(ti):
        t = tiles[ti]
        for b in range(cfg.n_blk):
            lo = H + b * BLK
            plo, zt = pick(t["zw"], lo)
            first = t["j"] == 0 and b == 0
            last = t["j"] == cfg.j_tiles - 1 and b == cfg.n_blk - 1
            nc.tensor.matmul(
                psums_z[t["pi"]][:], zt[:, lo - plo : lo - plo + BLK],
                rhs_for(t, b, False), start=first, stop=last
            )

    def e_mms(ti):
        t = tiles[ti]
        e2 = e2s[ti]
        blks = list(range(cfg.n_blk))
        if ti == nt - 1:  # issue in rev-scan segment order
            h2, h1 = cfg.n_blk // 2, cfg.n_blk // 4
            blks = (list(range(h2, cfg.n_blk)) + list(range(h1, h2))
                    + list(range(h1)))
        pieces = e2 if isinstance(e2, list) else [e2]
        for i, b in enumerate(blks):
            lo = H + b * BLK
            plo, et = pick(pieces, lo)
            first = t["j"] == 0 and i == 0
            last = t["j"] == cfg.j_tiles - 1 and i == cfg.n_blk - 1
            nc.tensor.matmul(
                psums_e[t["pi"]][:], et[:, lo - plo : lo - plo + BLK],
                rhs_for(t, b, True), start=first, stop=last
            )

    order = []
    for ti in range(nt):
        order.append(("z", ti))
        if ti >= 2:
            order.append(("e", ti - 2))
    order += [("e", nt - 2), ("e", nt - 1)]
    for kind, ti in order:
        (z_mms if kind == "z" else e_mms)(ti)


def build_nc(cfg: Cfg, split_waits=True, loop_n=1, unroll=1):
    """Build the per-core Bass program. Returns nc.

    loop_n > 1 wraps the body in an on-device For_i loop; unroll > 1
    replicates the body inline instead (for bench slope measurements).
    """
    import concourse.bass as bass
    import concourse.tile as tile
    import concourse.mybir as mybir

    f32 = mybir.dt.float32
    f16 = mybir.dt.float16

    P, TL, H = cfg.parts, cfg.tile_len, cfg.halo
    W = TL + 2 * H  # scan window length
    zdt = getattr(mybir.dt, cfg.z_dt)
    xdt = getattr(mybir.dt, cfg.x_dt)

    nc = bass.Bass()
    dram_in = {}
    for px, pz in PAIRS:
        dram_in[px] = nc.dram_tensor(px, [cfg.rows, cfg.T], xdt, kind="ExternalInput")
        # targets arrive host-staged as 2*z, padded with H zeros on each
        # side of every row
        dram_in[pz] = nc.dram_tensor(
            pz, [cfg.rows, cfg.T + 2 * cfg.halo], zdt, kind="ExternalInput"
        )
    n_acc = cfg.n_tiles + 1  # col per (pair, j) + tile-0 2nd piece
    acc_out = nc.dram_tensor("acc", [P, n_acc], f32, kind="ExternalOutput")
    # dots layout: [dz0(256) | de0(128) | dz1(256) | de1(128)] per partition
    dots_out = nc.dram_tensor(
        "dots", [cfg.blk, 6 * cfg.blk], f32, kind="ExternalOutput"
    )

    with tile.TileContext(nc) as tc:
        with (
            tc.tile_pool(name="const", bufs=1) as cpool,
            tc.tile_pool(name="zwin", bufs=4) as zpool,
            tc.tile_pool(name="gxp", bufs=4) as gpool,
            tc.tile_pool(name="efp", bufs=2) as epool,
            tc.tile_pool(name="e2p", bufs=3) as e2pool,
            tc.tile_pool(name="texp", bufs=cfg.texp_bufs) as tpool,
            tc.tile_pool(name="accp", bufs=1) as apool,
            tc.tile_pool(name="psum", bufs=1, space="PSUM") as ppool,
            tc.tile_pool(name="outp", bufs=1) as opool,
        ):
            # decay constant: either a single element per partition read
            # through a stride-0 broadcast AP, or a full-width tile written
            # as packed f32 pairs (half the memset cycles)
            if cfg.const_mode == "broadcast":
                const_v = cpool.tile([P, 1], f16, tag="cav")
                nc.vector.memset(const_v[:], DECAY)
            else:
                const_v = cpool.tile([P, W], f16, tag="cav")
                nc.vector.memset(const_v[:].bitcast(f32), DECAY_PAIR)

            acc = apool.tile([P, n_acc], f32, tag="acc")

            psums_z = [
                ppool.tile([cfg.blk, 2 * cfg.blk], f32, tag=f"pz{i}", name=f"pz{i}")
                for i in range(2)
            ]
            psums_e = [
                ppool.tile([cfg.blk, cfg.blk], f32, tag=f"pe{i}", name=f"pe{i}")
                for i in range(2)
            ]

            import contextlib

            loop_cm = (
                tc.For_i(0, loop_n, 1, hint_engines=(mybir.EngineType.PE,))
                if loop_n > 1
                else contextlib.nullcontext()
            )
            with loop_cm:
                for _ in range(unroll):
                    _build_body(nc, tc, cfg, dram_in, acc, psums_z, psums_e,
                                const_v, (zpool, gpool, epool, e2pool, tpool),
                                bass, mybir)

            # --- drain results on ACT (DVE is the critical path and
            # GPSIMD cannot access PSUM; ACT Copy reads PSUM fine). Each
            # drain gets its own slice of one tile and its own DMA so
            # early psum stops drain early.
            AF = mybir.ActivationFunctionType
            nc.sync.dma_start(acc_out[:], acc[:])
            dd = opool.tile([cfg.blk, 6 * cfg.blk], f32, tag="dots",
                            name="dots")
            off = 0
            for pi in range(2):
                nc.scalar.activation(
                    dd[:, off : off + 2 * cfg.blk], psums_z[pi][:], AF.Copy)
                nc.sync.dma_start(dots_out[:, off : off + 2 * cfg.blk],
                                  dd[:, off : off + 2 * cfg.blk])
                off += 2 * cfg.blk
                nc.scalar.activation(
                    dd[:, off : off + cfg.blk], psums_e[pi][:], AF.Copy)
                nc.sync.dma_start(dots_out[:, off : off + cfg.blk],
                                  dd[:, off : off + cfg.blk])
                off += cfg.blk

    if split_waits:
        _split_multiwaits(nc)
    return nc


def _split_multiwaits(nc):
    """Engine instructions hold at most ONE sync wait in core_v3 ISA structs
    (walrus: 'Too many sync wait commands'). Tile sometimes attaches 2+.
    Move extras onto same-engine NoOps inserted just before the instruction
    (sequencer executes them in order, so semantics are identical)."""
    import concourse.mybir as mybir

    for f in nc.m.functions:
        for blk in f.blocks:
            out = []
            changed = False
            for ins in blk.instructions:
                si = ins.sync_info
                cap = 2 if isinstance(ins, mybir.InstEventSemaphore) else 1
                if si is not None and si.on_wait and len(si.on_wait) > cap:
                    waits = list(si.on_wait)
                    for w in waits[:-cap]:
                        out.append(
                            mybir.InstNoOp(
                                name=nc.get_next_instruction_name(),
                                engine=ins.engine,
                                ins=[],
                                outs=[],
                                sync_info=mybir.SyncInfo(on_wait=[w], on_update=[]),
                            )
                        )
                    ins.sync_info = mybir.SyncInfo(
                        on_wait=waits[-cap:], on_update=list(si.on_update or [])
                    )
                    changed = True
                out.append(ins)
            if changed:
                blk.instructions = out


def host_combine(results, cfg: Cfg):
    """Combine per-core acc/dots into (start_loss, end_loss, total)."""
    n_elem = np.float64(B_FULL) * cfg.T
    losses = []
    B = cfg.blk
    for pi in range(2):
        s = np.float64(0.0)
        for res in results:
            acc = np.asarray(res["acc"], dtype=np.float64)
            dots = np.asarray(res["dots"], dtype=np.float64)
            o = pi * 3 * B
            dz = dots[:, o : o + 2 * B]
            de = dots[:, o + 2 * B : o + 3 * B]
            cols = [pi * cfg.j_tiles + j for j in range(cfg.j_tiles)]
            if pi == 0:
                cols.append(cfg.n_tiles)  # tile-0 second softplus piece
            s += acc[:, cols].sum()                      # sum(g)
            s += 0.5 * np.trace(de)                      # 0.5*sum(g*e2)
            s += np.trace(dz[:, 0:B])                    # sum(z2*g)
            s -= 2.0 * np.trace(dz[:, B : 2 * B])        # -2*sum(z2*x)
        losses.append(s / n_elem)
    start_loss, end_loss = losses
    total = (start_loss + end_loss) / 2.0
    return (
        np.float32(start_loss),
        np.float32(end_loss),
        np.float32(total),
    )


_NC_CACHE = {}
TRACE = False  # set True (e.g. from test.py) to capture an NTFF profile
LAST_RESULT = None  # BassKernelResults of the most recent run (for profiling)


def _np_dt(name):
    import ml_dtypes

    return {"float16": np.float16, "float8e4": ml_dtypes.float8_e4m3}[name]


def make_in_maps(cfg, inputs):
    """Host staging: shard rows, cast to the device dtypes, pad targets."""
    H = cfg.halo
    xnp, znp = _np_dt(cfg.x_dt), _np_dt(cfg.z_dt)
    in_maps = []
    for k in range(N_CORES):
        rs = slice(k * ROWS, (k + 1) * ROWS)
        m = {}
        for px, pz in PAIRS:
            m[px] = np.ascontiguousarray(np.asarray(inputs[px])[rs]).astype(xnp)
            z2p = np.zeros((ROWS, cfg.T + 2 * H), dtype=znp)
            # targets are exactly 0.0/1.0; 2*z is exact in fp16/fp8
            z2p[:, H : H + cfg.T] = (np.asarray(inputs[pz])[rs] * 2.0).astype(znp)
            m[pz] = z2p
        in_maps.append(m)
    return in_maps


def kernel(**inputs):
    from concourse.bass_utils import run_bass_kernel_spmd

    cfg = PROD_CFG
    key = "prod"
    if key not in _NC_CACHE:
        _NC_CACHE[key] = build_nc(cfg)
    nc = _NC_CACHE[key]

    in_maps = make_in_maps(cfg, inputs)
    res = run_bass_kernel_spmd(
        nc, in_maps, core_ids=list(range(N_CORES)), trace=TRACE
    )
    global LAST_RESULT
    LAST_RESULT = res
    return host_combine(res.results, cfg)


# revision 32
# speedup vs baseline: 2.6726x; 2.6726x over previous
"""Trainium2 Bass kernel for nn_BoundaryDetectionLoss.

Computes, for start/end (probs, targets) pairs of shape (64, 131072):
    w   = 1 + exp(-dist_to_nearest_boundary / 5)     (distance transform)
    bce = (1-z)*x + (1+z)*softplus(-x)               (pos_weight = 2)
    loss = mean(bce * w)   per pair; total = (start_loss + end_loss)/2

Identity used on device (g = softplus(+x), e = exp(-dist/5), z*e == z):
    bce*w = g*(1 + e + 2z) - 4*z*x
so with host-staged z2 = 2*z (zero-padded halo) and e2 = 2e from the
decayed-max scans:
    sum(bce*w) = sum(g) + 0.5*sum(g*e2) + sum(z2*g) - 2*sum(z2*x)

Device algorithm (per core, data-parallel over 8 rows of B=64):
  - e2[t] = 2*exp(-dist[t]/5) as a decayed-max field with two DVE
    tensor_tensor_scan passes per tile (op0=mult by a=exp(-1/5), op1=max),
    64-element halo (contributions beyond ~38 positions underflow below the
    fp8 output's subnormal floor, and beyond ~84 below fp16 ulp(1)).
    Scans are DVE-only: TensorTensorScanArith is not a legal GPSIMD opcode,
    and the decayed-max recurrence is inherently 2 passes x 1 elem/cycle on
    the 128-lane DVE, making ~34us the per-core scan floor for this shape.
  - g = softplus(x) = ln(1+exp(x)) on ACT (Exp+Ln share one LUT set; walrus
    has no softplus set); the Ln's accum_out gives sum(g) per partition free.
  - Dots sum(z2*g), sum(z2*x), sum(e2*g) on the PE via 128-wide block
    matmuls accumulating lhsT^T @ rhs into PSUM; ACT (which can read PSUM;
    GPSIMD cannot) copies the results out and the host sums the block
    diagonals.
  - Inputs are staged fp8-e4m3 by the host (pure dtype conversion + x2
    scaling + padding): 0/2 targets are exact in fp8, x/g quantization
    noise averages out far below the 2e-2 gate (measured 2.9e-4), and HBM
    traffic drops 4x vs f32 (4.3MB/core).
  - The pipeline-head tile and the pipeline-tail reverse scan are split
    into separate piece TILES (Tile tracks hazards per tile, not per AP
    range) so the scan chain starts after a fraction of the first DMA and
    the final e-matmuls unblock segment by segment.
"""

import sys

for _p in ("/opt/trn_rl_repo", "/root/.axon_site/_ro/trn_rl_repo"):
    if _p not in sys.path:
        sys.path.append(_p)

import numpy as np

# ---------------------------------------------------------------- config
B_FULL = 64
T_FULL = 131072
N_CORES = 8
ROWS = B_FULL // N_CORES  # 8 rows per core
DECAY = float(np.float16(np.exp(np.float32(-0.2))))  # a = exp(-1/5) in fp16
# two fp16 DECAY values bit-packed as one f32 (memset the const tile at
# half the DVE cycles by writing f32 pairs)
DECAY_PAIR = float(
    np.frombuffer(np.array([DECAY, DECAY], np.float16).tobytes(), np.float32)[0]
)


class Cfg:
    def __init__(self, rows=8, chunks=16, j_tiles=2, tile_len=4096, halo=64,
                 z_dt="float8e4", x_dt="float8e4", e2_dt="float8e4",
                 const_mode="broadcast", texp_bufs=2,
                 zw_bufs=4, head_bufs=2):
        self.rows = rows
        self.chunks = chunks
        self.j_tiles = j_tiles
        self.tile_len = tile_len
        self.halo = halo
        self.chunk_len = j_tiles * tile_len
        self.T = chunks * self.chunk_len
        self.parts = rows * chunks
        assert self.parts <= 128
        self.blk = 128
        self.n_blk = tile_len // self.blk
        assert halo <= tile_len
        self.z_dt = z_dt
        self.x_dt = x_dt
        self.e2_dt = e2_dt
        self.const_mode = const_mode
        self.texp_bufs = texp_bufs
        self.zw_bufs = zw_bufs
        self.head_bufs = head_bufs
        self.n_tiles = 2 * j_tiles


PROD_CFG = Cfg()
PAIRS = (("start_probs", "start_targets"), ("end_probs", "end_targets"))


def _build_body(nc, tc, cfg, dram_in, acc, psums_z, psums_e,
                const_v, pools, bass, mybir):
    f16 = mybir.dt.float16
    AF = mybir.ActivationFunctionType
    OP = mybir.AluOpType
    zpool, gpool, epool, e2pool, tpool, hpool = pools
    P, TL, H = cfg.parts, cfg.tile_len, cfg.halo
    W = TL + 2 * H
    zdt = getattr(mybir.dt, cfg.z_dt)
    xdt = getattr(mybir.dt, cfg.x_dt)
    e2dt = getattr(mybir.dt, cfg.e2_dt)
    Tp = cfg.T + 2 * H  # padded row length
    nt = cfg.n_tiles
    BLK = cfg.blk

    # Tile 0 is the pipeline head: its loads/softplus/scan are split into
    # pieces (SEPARATE tiles — Tile tracks hazards per tile, not per range)
    # so the DVE scan chain and the ACT chain both start as early as
    # possible. Piece boundaries are block-aligned for the PE slices.
    ZW0_CUTS = (0, H + 4 * BLK, H + 16 * BLK, W)
    GX0_CUTS = (0, TL // 2, TL)

    # ---- phase 1: DMA loads + ACT softplus, tile-major (zw first: the DVE
    # scan chain is the critical path and consumes zw earliest)
    tiles = []  # per tile: dict(zw=[(lo,hi,tile)], gx=[(tlo,thi,tile,acc_col)])
    for pi, (px, pz) in enumerate(PAIRS):
        xd, zd = dram_in[px], dram_in[pz]
        x4 = xd[:].rearrange(
            "r (c j f) -> (r c) j f", c=cfg.chunks, j=cfg.j_tiles
        )
        for j in range(cfg.j_tiles):
            ti = pi * cfg.j_tiles + j
            zw_cuts = ZW0_CUTS if ti == 0 else (0, W)
            gx_cuts = GX0_CUTS if ti == 0 else (0, TL)

            zw = []
            for si in range(len(zw_cuts) - 1):
                lo, hi = zw_cuts[si], zw_cuts[si + 1]
                zp = hpool if ti == 0 else zpool
                zt = zp.tile([P, hi - lo], zdt, tag=f"zw{min(ti,1)}{si}",
                             name=f"zw{min(ti,1)}{si}")
                zw.append((lo, hi, zt))

            # gx piece = [g | x] halves: DMA x into the right half, ACT
            # writes g = softplus(x) = ln(1 + exp(x)) into the left half,
            # so one 256-wide PE moving operand covers both z2@g and z2@x.
            # (No softplus LUT set exists in walrus; Exp+Ln share one set.)
            gx = []
            for si in range(len(gx_cuts) - 1):
                tlo, thi = gx_cuts[si], gx_cuts[si + 1]
                n = thi - tlo
                gp = hpool if ti == 0 else gpool
                gt = gp.tile([P, 2 * n], xdt, tag=f"gx{min(ti,1)}{si}",
                             name=f"gx{min(ti,1)}{si}")
                gx.append((tlo, thi, gt))

            # DMA issue order: first zw piece, then first x piece (unblocks
            # the ACT chain), then the rest
            def _dma_z(si):
                lo, hi, zt = zw[si]
                zwin = bass.AP(
                    zd,
                    j * TL + lo,
                    [[Tp, cfg.rows], [cfg.chunk_len, cfg.chunks],
                     [1, hi - lo]],
                )
                nc.sync.dma_start(zt[:], zwin)

            def _dma_x(si):
                tlo, thi, gt = gx[si]
                n = thi - tlo
                nc.sync.dma_start(gt[:, n : 2 * n], x4[:, j, tlo:thi])

            for si in range(len(zw)):
                _dma_z(si)
            for si in range(len(gx)):
                _dma_x(si)

            for si, (tlo, thi, gt) in enumerate(gx):
                n = thi - tlo
                tp = hpool if ti == 0 else tpool
                texp = tp.tile([P, n], f16, tag=f"texp{min(ti,1)}{si}",
                               name=f"texp{min(ti,1)}{si}")
                nc.scalar.activation(texp[:], gt[:, n : 2 * n], AF.Exp)
                col = ti if si == 0 else nt + si - 1  # extra accum cols
                nc.scalar.activation(
                    gt[:, 0:n], texp[:], AF.Ln, bias=1.0,
                    accum_out=acc[:, col : col + 1],
                )
            tiles.append(dict(pi=pi, j=j, zw=zw, gx=gx))

    # ---- phase 2: DVE scans (fwd full window in chained piece segments,
    # rev only [H, W) reversed). Scans carry at most one ISA sync wait;
    # _split_multiwaits moves any extras Tile attaches onto same-engine
    # NoOps. The LAST tile's reverse scan lands in three chained segment
    # tiles so its e-matmuls (the tail of the whole kernel) unblock chunk
    # by chunk.
    if cfg.const_mode == "broadcast":
        cb = lambda n: const_v[:].broadcast_to((P, n))  # noqa: E731
    else:
        cb = lambda n: const_v[:, 0:n]  # noqa: E731 (value-constant tile)
    e2s = []
    for ti, t in enumerate(tiles):
        ef = epool.tile([P, W], f16, tag="ef")
        prev_hi = None
        for lo, hi, zt in t["zw"]:
            init = 0.0 if prev_hi is None else ef[:, lo - 1 : lo]
            nc.vector.tensor_tensor_scan(
                ef[:, lo:hi], cb(hi - lo), zt[:], init, OP.mult, OP.max)
            prev_hi = hi
        if ti == nt - 1:
            m2 = H + TL // 2
            m1 = H + TL // 4
            segs = []
            prev = None
            for si, (lo, hi) in enumerate(((m2, W), (m1, m2), (H, m1))):
                st = hpool.tile([P, hi - lo], e2dt, tag=f"e2s{si}",
                                name=f"e2s{si}")
                init = 0.0 if prev is None else prev[:, 0:1]
                nc.vector.tensor_tensor_scan(
                    st[:, ::-1], cb(hi - lo),
                    ef[:, hi - 1 : lo - 1 : -1], init, OP.mult, OP.max
                )
                segs.append((lo, hi, st))
                prev = st
            e2s.append(segs)
        else:
            # tile local coord k holds window position H+k
            e2 = e2pool.tile([P, W - H], e2dt, tag="e2")
            nc.vector.tensor_tensor_scan(
                e2[:, ::-1], cb(W - H),
                ef[:, W - 1 : H - 1 : -1], 0.0, OP.mult, OP.max
            )
            e2s.append((H, W, e2))

    # ---- phase 3: PE matmuls. z-mms of a tile depend only on (zw, gx);
    # e-mms additionally on that tile's rev scan. Order z(0), z(1), e(0),
    # z(2), e(1), z(3), e(2), e(3) keeps the PE fed while scans complete.
    def pick(pieces, lo):
        for plo, phi, pt in pieces:
            if plo <= lo < phi:
                return plo, pt
        raise AssertionError(f"no piece for {lo}")

    def rhs_for(t, b, g_only):
        tpos = b * BLK
        tlo, gt = pick(t["gx"], tpos)
        n = gt.shape[1] // 2
        o = tpos - tlo
        if g_only:
            return gt[:, o : o + BLK]
        g3 = gt[:].rearrange("p (g f) -> p g f", g=2)
        return g3[:, :, o : o + BLK]

    def z_mms(ti):
        t = tiles[ti]
        for b in range(cfg.n_blk):
            lo = H + b * BLK
            plo, zt = pick(t["zw"], lo)
            first = t["j"] == 0 and b == 0
            last = t["j"] == cfg.j_tiles - 1 and b == cfg.n_blk - 1
            nc.tensor.matmul(
                psums_z[t["pi"]][:], zt[:, lo - plo : lo - plo + BLK],
                rhs_for(t, b, False), start=first, stop=last
            )

    def e_mms(ti):
        t = tiles[ti]
        e2 = e2s[ti]
        blks = list(range(cfg.n_blk))
        if ti == nt - 1:  # issue in rev-scan segment order
            h2, h1 = cfg.n_blk // 2, cfg.n_blk // 4
            blks = (list(range(h2, cfg.n_blk)) + list(range(h1, h2))
                    + list(range(h1)))
        pieces = e2 if isinstance(e2, list) else [e2]
        for i, b in enumerate(blks):
            lo = H + b * BLK
            plo, et = pick(pieces, lo)
            first = t["j"] == 0 and i == 0
            last = t["j"] == cfg.j_tiles - 1 and i == cfg.n_blk - 1
            nc.tensor.matmul(
                psums_e[t["pi"]][:], et[:, lo - plo : lo - plo + BLK],
                rhs_for(t, b, True), start=first, stop=last
            )

    order = []
    for ti in range(nt):
        order.append(("z", ti))
        if ti >= 2:
            order.append(("e", ti - 2))
    order += [("e", nt - 2), ("e", nt - 1)]
    for kind, ti in order:
        (z_mms if kind == "z" else e_mms)(ti)


def build_nc(cfg: Cfg, split_waits=True, loop_n=1, unroll=1):
    """Build the per-core Bass program. Returns nc.

    loop_n > 1 wraps the body in an on-device For_i loop; unroll > 1
    replicates the body inline instead (for bench slope measurements).
    """
    import concourse.bass as bass
    import concourse.tile as tile
    import concourse.mybir as mybir

    f32 = mybir.dt.float32
    f16 = mybir.dt.float16

    P, TL, H = cfg.parts, cfg.tile_len, cfg.halo
    W = TL + 2 * H  # scan window length
    zdt = getattr(mybir.dt, cfg.z_dt)
    xdt = getattr(mybir.dt, cfg.x_dt)

    nc = bass.Bass()
    dram_in = {}
    for px, pz in PAIRS:
        dram_in[px] = nc.dram_tensor(px, [cfg.rows, cfg.T], xdt, kind="ExternalInput")
        # targets arrive host-staged as 2*z, padded with H zeros on each
        # side of every row
        dram_in[pz] = nc.dram_tensor(
            pz, [cfg.rows, cfg.T + 2 * cfg.halo], zdt, kind="ExternalInput"
        )
    n_acc = cfg.n_tiles + 1  # col per (pair, j) + tile-0 2nd piece
    acc_out = nc.dram_tensor("acc", [P, n_acc], f32, kind="ExternalOutput")
    # dots layout: [dz0(256) | de0(128) | dz1(256) | de1(128)] per partition
    dots_out = nc.dram_tensor(
        "dots", [cfg.blk, 6 * cfg.blk], f32, kind="ExternalOutput"
    )

    with tile.TileContext(nc) as tc:
        with (
            tc.tile_pool(name="const", bufs=1) as cpool,
            tc.tile_pool(name="zwin", bufs=cfg.zw_bufs) as zpool,
            tc.tile_pool(name="gxp", bufs=4) as gpool,
            tc.tile_pool(name="efp", bufs=2) as epool,
            tc.tile_pool(name="e2p", bufs=3) as e2pool,
            tc.tile_pool(name="texp", bufs=cfg.texp_bufs) as tpool,
            tc.tile_pool(name="head", bufs=cfg.head_bufs) as hpool,
            tc.tile_pool(name="accp", bufs=1) as apool,
            tc.tile_pool(name="psum", bufs=1, space="PSUM") as ppool,
            tc.tile_pool(name="outp", bufs=1) as opool,
        ):
            # decay constant: either a single element per partition read
            # through a stride-0 broadcast AP, or a full-width tile written
            # as packed f32 pairs (half the memset cycles)
            if cfg.const_mode == "broadcast":
                const_v = cpool.tile([P, 1], f16, tag="cav")
                nc.vector.memset(const_v[:], DECAY)
            else:
                const_v = cpool.tile([P, W], f16, tag="cav")
                nc.vector.memset(const_v[:].bitcast(f32), DECAY_PAIR)

            acc = apool.tile([P, n_acc], f32, tag="acc")

            psums_z = [
                ppool.tile([cfg.blk, 2 * cfg.blk], f32, tag=f"pz{i}", name=f"pz{i}")
                for i in range(2)
            ]
            psums_e = [
                ppool.tile([cfg.blk, cfg.blk], f32, tag=f"pe{i}", name=f"pe{i}")
                for i in range(2)
            ]

            import contextlib

            loop_cm = (
                tc.For_i(0, loop_n, 1, hint_engines=(mybir.EngineType.PE,))
                if loop_n > 1
                else contextlib.nullcontext()
            )
            with loop_cm:
                for _ in range(unroll):
                    _build_body(nc, tc, cfg, dram_in, acc, psums_z, psums_e,
                                const_v,
                                (zpool, gpool, epool, e2pool, tpool, hpool),
                                bass, mybir)

            # --- drain results on ACT (DVE is the critical path and
            # GPSIMD cannot access PSUM; ACT Copy reads PSUM fine). Each
            # drain gets its own slice of one tile and its own DMA so
            # early psum stops drain early.
            AF = mybir.ActivationFunctionType
            nc.sync.dma_start(acc_out[:], acc[:])
            dd = opool.tile([cfg.blk, 6 * cfg.blk], f32, tag="dots",
                            name="dots")
            off = 0
            for pi in range(2):
                nc.scalar.activation(
                    dd[:, off : off + 2 * cfg.blk], psums_z[pi][:], AF.Copy)
                nc.sync.dma_start(dots_out[:, off : off + 2 * cfg.blk],
                                  dd[:, off : off + 2 * cfg.blk])
                off += 2 * cfg.blk
                nc.scalar.activation(
                    dd[:, off : off + cfg.blk], psums_e[pi][:], AF.Copy)
                nc.sync.dma_start(dots_out[:, off : off + cfg.blk],
                                  dd[:, off : off + cfg.blk])
                off += cfg.blk

    if split_waits:
        _split_multiwaits(nc)
    return nc


def _split_multiwaits(nc):
    """Engine instructions hold at most ONE sync wait in core_v3 ISA structs
    (walrus: 'Too many sync wait commands'). Tile sometimes attaches 2+.
    Move extras onto same-engine NoOps inserted just before the instruction
    (sequencer executes them in order, so semantics are identical)."""
    import concourse.mybir as mybir

    for f in nc.m.functions:
        for blk in f.blocks:
            out = []
            changed = False
            for ins in blk.instructions:
                si = ins.sync_info
                cap = 2 if isinstance(ins, mybir.InstEventSemaphore) else 1
                if si is not None and si.on_wait and len(si.on_wait) > cap:
                    waits = list(si.on_wait)
                    for w in waits[:-cap]:
                        out.append(
                            mybir.InstNoOp(
                                name=nc.get_next_instruction_name(),
                                engine=ins.engine,
                                ins=[],
                                outs=[],
                                sync_info=mybir.SyncInfo(on_wait=[w], on_update=[]),
                            )
                        )
                    ins.sync_info = mybir.SyncInfo(
                        on_wait=waits[-cap:], on_update=list(si.on_update or [])
                    )
                    changed = True
                out.append(ins)
            if changed:
                blk.instructions = out


def host_combine(results, cfg: Cfg):
    """Combine per-core acc/dots into (start_loss, end_loss, total)."""
    n_elem = np.float64(B_FULL) * cfg.T
    losses = []
    B = cfg.blk
    for pi in range(2):
        s = np.float64(0.0)
        for res in results:
            acc = np.asarray(res["acc"], dtype=np.float64)
            dots = np.asarray(res["dots"], dtype=np.float64)
            o = pi * 3 * B
            dz = dots[:, o : o + 2 * B]
            de = dots[:, o + 2 * B : o + 3 * B]
            cols = [pi * cfg.j_tiles + j for j in range(cfg.j_tiles)]
            if pi == 0:
                cols.append(cfg.n_tiles)  # tile-0 second softplus piece
            s += acc[:, cols].sum()                      # sum(g)
            s += 0.5 * np.trace(de)                      # 0.5*sum(g*e2)
            s += np.trace(dz[:, 0:B])                    # sum(z2*g)
            s -= 2.0 * np.trace(dz[:, B : 2 * B])        # -2*sum(z2*x)
        losses.append(s / n_elem)
    start_loss, end_loss = losses
    total = (start_loss + end_loss) / 2.0
    return (
        np.float32(start_loss),
        np.float32(end_loss),
        np.float32(total),
    )


_NC_CACHE = {}
TRACE = False  # set True (e.g. from test.py) to capture an NTFF profile
LAST_RESULT = None  # BassKernelResults of the most recent run (for profiling)


def _np_dt(name):
    import ml_dtypes

    return {"float16": np.float16, "float8e4": ml_dtypes.float8_e4m3}[name]


def make_in_maps(cfg, inputs):
    """Host staging: shard rows, cast to the device dtypes, pad targets."""
    H = cfg.halo
    xnp, znp = _np_dt(cfg.x_dt), _np_dt(cfg.z_dt)
    in_maps = []
    for k in range(N_CORES):
        rs = slice(k * ROWS, (k + 1) * ROWS)
        m = {}
        for px, pz in PAIRS:
            m[px] = np.ascontiguousarray(np.asarray(inputs[px])[rs]).astype(xnp)
            z2p = np.zeros((ROWS, cfg.T + 2 * H), dtype=znp)
            # targets are exactly 0.0/1.0; 2*z is exact in fp16/fp8
            z2p[:, H : H + cfg.T] = (np.asarray(inputs[pz])[rs] * 2.0).astype(znp)
            m[pz] = z2p
        in_maps.append(m)
    return in_maps


def kernel(**inputs):
    from concourse.bass_utils import run_bass_kernel_spmd

    cfg = PROD_CFG
    key = "prod"
    if key not in _NC_CACHE:
        _NC_CACHE[key] = build_nc(cfg)
    nc = _NC_CACHE[key]

    in_maps = make_in_maps(cfg, inputs)
    res = run_bass_kernel_spmd(
        nc, in_maps, core_ids=list(range(N_CORES)), trace=TRACE
    )
    global LAST_RESULT
    LAST_RESULT = res
    return host_combine(res.results, cfg)
